# revision 3
# baseline (speedup 1.0000x reference)
"""Trainium2 Bass kernel for LocalDenseConv2D + BatchNorm + PReLU.

Problem (hardcoded shapes):
  x:      (8, 64, 64, 256)  f32   (B, IN_C, L, T)
  weight: (576, 64, 64)     f32   (K = IN_C*9, OUT_C, OUT_L)  k = ci*9 + di*3 + dj
  bias:   (64, 64)          f32   (OUT_C, OUT_L)
  gamma, beta: (64,)        f32
  alpha:  (1,)              f32
  out:    (8, 64, 64, 256)  f32

Sharding: out_l across 8 cores (8 rows each), all batches per core.
Inputs are converted to bf16 on the host (matmul runs 1 cycle/row vs 2 for
f32r-HIGH, and input DMA halves).  Two conv structures:

  "k128": contraction packed to K=128 by stacking taps (di=0,di=1) on the
    two partition halves of a row-shifted x copy; taps di=2 run as K=64
    matmuls on partitions 0-63.  6 matmuls per (lp, bh, nb) region;
    98304 psum rows per core.  PSUM holds (bh*64+c) x (nb, b4, t).

  "quad4": baseline tap structure (K=64 over ci) but all four 64x64 PE
    quadrants active: row groups = batch half (bh), col groups = b4 pair
    (nt).  147456 rows but up to 4-way quadrant concurrency.

BatchNorm: per-partition bn_stats/bn_aggr -> (sum, sumsq) -> AllGather
[128,2] -> gather both partition halves per channel -> reduce -> exact
global mean/var.  BN-apply + PReLU is one scalar-engine Prelu with
per-partition scale/bias; the bias pass also uses Prelu (alpha=1) so only
one activation table set is ever loaded.  Output stored bf16, upcast on
host.
"""
import sys
import numpy as np

if '/opt/trn_rl_repo' not in sys.path:
    sys.path.insert(0, '/opt/trn_rl_repo')

import concourse.bass as bass
import concourse.bacc as bacc
import concourse.mybir as mybir
import concourse.tile as tile
from concourse.bass_utils import run_bass_kernel_spmd

import ml_dtypes

BF16_NP = ml_dtypes.bfloat16
F32 = mybir.dt.float32
BF16 = mybir.dt.bfloat16
AF = mybir.ActivationFunctionType
ALU = mybir.AluOpType

B, IN_C, L, T = 8, 64, 64, 256
OUT_C, OUT_L = 64, 64
NCORES = 8
L_LOC = L // NCORES          # 8 out_l rows per core
SLAB = L_LOC + 2             # 10 x-rows incl. halo
TP = T + 2                   # padded t
EPS = 1e-5
N_PART = L_LOC * 2 * T * 2   # elems per partition per core = 8192
N_GLOBAL = B * L * T         # 131072

VARIANT = "k128"             # "k128" or "quad4"

_cache = {}


def _build_common_tail(nc, tc, cpool, ott, stats, gt, et, att, epst, yo):
    """stats merge -> collective -> global scale/shift -> final pass."""
    cc_in = nc.dram_tensor("cc_in", [128, 2], F32)
    cc_out = nc.dram_tensor("cc_out", [NCORES * 128, 2], F32,
                            addr_space="Shared")

    # local (mean, var) -> (sum, sumsq); off the critical path
    loc = cpool.tile([128, 2], F32)
    nc.vector.bn_aggr(loc[:], stats[:].rearrange("p a b c -> p (a b c)"))
    msq = cpool.tile([128, 1], F32)
    t2 = cpool.tile([128, 1], F32)
    ccs = cpool.tile([128, 2], F32)
    nc.vector.tensor_tensor(msq[:], loc[:, 0:1], loc[:, 0:1], ALU.mult)
    nc.vector.tensor_tensor(t2[:], loc[:, 1:2], msq[:], ALU.add)
    nc.vector.tensor_scalar_mul(ccs[:, 0:1], loc[:, 0:1], float(N_PART))
    nc.vector.tensor_scalar_mul(ccs[:, 1:2], t2[:], float(N_PART))
    nc.sync.dma_start(cc_in.ap(), ccs[:])
    nc.gpsimd.collective_compute(
        "AllGather", ALU.bypass,
        replica_groups=[list(range(NCORES))],
        ins=[cc_in[:]], outs=[cc_out[:]])

    # gather: partition p gets all 16 (rank, half) partials of its channel
    gm = cpool.tile([128, 2, 16], F32)
    src = cc_out.ap().rearrange("(r q c) s -> c s (r q)", r=NCORES, q=2, c=64)
    nc.sync.dma_start(gm[0:64, :, :], src)
    nc.scalar.dma_start(gm[64:128, :, :], src)

    tot = cpool.tile([128, 2], F32)
    nc.vector.tensor_reduce(tot[:], gm[:, :, :], axis=mybir.AxisListType.X,
                            op=ALU.add)
    mv = cpool.tile([128, 2], F32)       # (mean, E[x^2])
    nc.vector.tensor_scalar_mul(mv[:], tot[:], 1.0 / N_GLOBAL)
    msq2 = cpool.tile([128, 1], F32)
    var = cpool.tile([128, 1], F32)
    nc.vector.tensor_tensor(msq2[:], mv[:, 0:1], mv[:, 0:1], ALU.mult)
    nc.vector.tensor_tensor(var[:], mv[:, 1:2], msq2[:], ALU.subtract)
    std = cpool.tile([128, 1], F32)
    rstd = cpool.tile([128, 1], F32)
    sca = cpool.tile([128, 1], F32)
    shi = cpool.tile([128, 1], F32)
    nc.scalar.activation(std[:], var[:], AF.Sqrt, bias=epst[:])
    nc.vector.reciprocal(rstd[:], std[:])
    nc.vector.tensor_tensor(sca[:], gt[:], rstd[:], ALU.mult)
    nc.vector.tensor_tensor(shi[:], mv[:, 0:1], sca[:], ALU.mult)
    nc.vector.tensor_tensor(shi[:], et[:], shi[:], ALU.subtract)

    # fused BN-apply + PReLU + store (bf16)
    with tc.tile_pool(name="fp", bufs=3) as fpool:
        out_q = [nc.sync, nc.gpsimd]
        for lp in range(L_LOC):
            fo = fpool.tile([128, 1024], BF16, tag="fo")
            nc.scalar.activation(fo[:], ott[:, lp, :], AF.Prelu,
                                 bias=shi[:], scale=sca[:], alpha=att[:])
            out_q[lp % 2].dma_start(yo.ap()[:, lp, :], fo[:])


def _build(variant):
    nc = bacc.Bacc("TRN2", target_bir_lowering=False, debug=False,
                   num_devices=NCORES)
    bt_d = nc.dram_tensor("bt", [128, L_LOC], F32, kind="ExternalInput")
    g_d = nc.dram_tensor("g", [128, 1], F32, kind="ExternalInput")
    e_d = nc.dram_tensor("e", [128, 1], F32, kind="ExternalInput")
    a_d = nc.dram_tensor("a", [128, 1], F32, kind="ExternalInput")
    yo = nc.dram_tensor("yo", [128, L_LOC, 1024], BF16, kind="ExternalOutput")

    if variant == "k128":
        xa_d = nc.dram_tensor("xa", [128, B, SLAB, TP], BF16,
                              kind="ExternalInput")
        wA_d = nc.dram_tensor("wA", [128, 3, L_LOC, OUT_C], BF16,
                              kind="ExternalInput")
        wC_d = nc.dram_tensor("wC", [64, 3, L_LOC, OUT_C], BF16,
                              kind="ExternalInput")
    else:
        xa_d = nc.dram_tensor("xa", [128, 4, SLAB, TP], BF16,
                              kind="ExternalInput")
        wA_d = nc.dram_tensor("wA", [128, 9, L_LOC, OUT_C], BF16,
                              kind="ExternalInput")

    with tile.TileContext(nc) as tc:
        with (
            tc.tile_pool(name="const", bufs=1) as cpool,
            tc.tile_pool(name="xp", bufs=1) as xpool,
            tc.tile_pool(name="op", bufs=1) as opool,
            tc.tile_pool(name="ps", bufs=2, space="PSUM") as ppool,
        ):
            bt = cpool.tile([128, L_LOC], F32)
            gt = cpool.tile([128, 1], F32)
            et = cpool.tile([128, 1], F32)
            att = cpool.tile([128, 1], F32)
            onet = cpool.tile([128, 1], F32)
            epst = cpool.tile([128, 1], F32)
            nc.vector.memset(onet[:], 1.0)
            nc.vector.memset(epst[:], EPS)

            ott = opool.tile([128, L_LOC, 1024], F32)
            stats = cpool.tile([128, L_LOC, 2, 6], F32)

            if variant == "k128":
                wA = cpool.tile([128, 3, L_LOC, OUT_C], BF16)
                wC = cpool.tile([64, 3, L_LOC, OUT_C], BF16)
                xa = xpool.tile([128, B, SLAB, TP], BF16)
                nc.sync.dma_start(wA[:], wA_d.ap())
                nc.sync.dma_start(wC[:], wC_d.ap())
            else:
                wA = cpool.tile([128, 9, L_LOC, OUT_C], BF16)
                xa = xpool.tile([128, 4, SLAB, TP], BF16)
                nc.sync.dma_start(wA[:], wA_d.ap())
            nc.sync.dma_start(bt[:], bt_d.ap())
            nc.sync.dma_start(gt[:], g_d.ap())
            nc.sync.dma_start(et[:], e_d.ap())
            nc.sync.dma_start(att[:], a_d.ap())
            # x rows on two queues, in row order so early lps unblock first
            for s in range(SLAB):
                q = nc.sync if s % 2 == 0 else nc.scalar
                q.dma_start(xa[:, :, s, :], xa_d.ap()[:, :, s, :])

            # ---- conv ----
            for lp in range(L_LOC):
                pt = ppool.tile([128, 1024], F32, tag="pt")
                if variant == "k128":
                    # region (bh, nb): batches bh*4+2nb+{0,1}
                    for nb in range(2):
                        for mi in range(6):
                            first = mi == 0
                            last = mi == 5
                            for bh in range(2):
                                bsel = bh * 4 + 2 * nb
                                if mi < 3:   # K=128 pair (di=0,1), dj=mi
                                    j = mi
                                    lhsT = wA[:, j, lp, :]
                                    rhs = xa[:, bsel:bsel + 2, lp, j:j + T]
                                else:        # K=64 single di=2, dj=mi-3
                                    j = mi - 3
                                    lhsT = wC[:, j, lp, :]
                                    rhs = xa[0:64, bsel:bsel + 2, lp + 2,
                                             j:j + T]
                                nc.tensor.matmul(
                                    pt[bh * 64:(bh + 1) * 64,
                                       nb * 512:(nb + 1) * 512],
                                    lhsT, rhs, start=first, stop=last)
                else:
                    # quad4: psum partition = nt*64+c, free = bh*512+...
                    for combo in range(9):
                        di, dj = combo // 3, combo % 3
                        first = combo == 0
                        last = combo == 8
                        for bh in range(2):
                            for nt in range(2):
                                lhsT = wA[bh * 64:(bh + 1) * 64, combo, lp, :]
                                rhs = xa[bh * 64:(bh + 1) * 64,
                                         2 * nt:2 * nt + 2, lp + di,
                                         dj:dj + T]
                                nc.tensor.matmul(
                                    pt[nt * 64:(nt + 1) * 64,
                                       bh * 512:(bh + 1) * 512],
                                    lhsT, rhs, start=first, stop=last)

                # bias + copy to SBUF (Prelu with alpha=1 == identity)
                nc.scalar.activation(ott[:, lp, :], pt[:, :], AF.Prelu,
                                     bias=bt[:, lp:lp + 1], alpha=onet[:])
                for h in range(2):
                    nc.vector.bn_stats(stats[:, lp, h, :],
                                       pt[:, h * 512:(h + 1) * 512])

            _build_common_tail(nc, tc, cpool, ott, stats, gt, et, att,
                               epst, yo)
    nc.compile()
    return nc


def _prep(x, weight, bias, gamma, beta, alpha, variant):
    """Build per-core input maps (host-side shard + relayout, bf16)."""
    xpad = np.zeros((B, IN_C, L + 3, TP + 1), np.float32)
    xpad[:, :, 1:L + 1, 1:T + 1] = x
    xpad = xpad.astype(BF16_NP)
    wl = weight.reshape(IN_C, 3, 3, OUT_C, OUT_L).astype(BF16_NP)

    in_maps = []
    for r in range(NCORES):
        l0 = r * L_LOC
        m = {
            "bt": np.ascontiguousarray(
                np.tile(bias[:, l0:l0 + L_LOC], (2, 1))).astype(np.float32),
            "g": np.tile(gamma.reshape(-1, 1), (2, 1)).astype(np.float32),
            "e": np.tile(beta.reshape(-1, 1), (2, 1)).astype(np.float32),
            "a": np.full((128, 1), float(alpha[0]), np.float32),
        }
        if variant == "k128":
            slab = xpad[:, :, l0:l0 + SLAB + 1, :TP]  # (B, C, 11, 258)
            xa = np.empty((128, B, SLAB, TP), BF16_NP)
            xa[0:64] = slab[:, :, 0:SLAB].transpose(1, 0, 2, 3)
            xa[64:128] = slab[:, :, 1:SLAB + 1].transpose(1, 0, 2, 3)
            wv = wl[:, :, :, :, l0:l0 + L_LOC]  # (ci, di, dj, c, lp)
            wA = np.empty((128, 3, L_LOC, OUT_C), BF16_NP)
            wA[0:64] = wv[:, 0].transpose(0, 1, 3, 2)   # (ci, dj, lp, c)
            wA[64:128] = wv[:, 1].transpose(0, 1, 3, 2)
            wC = np.ascontiguousarray(wv[:, 2].transpose(0, 1, 3, 2))
            m.update(xa=xa, wA=wA, wC=np.ascontiguousarray(wC))
        else:
            slab = xpad[:, :, l0:l0 + SLAB, :TP]       # (B, C, 10, 258)
            xa = np.ascontiguousarray(
                slab.reshape(2, 4, IN_C, SLAB, TP).transpose(0, 2, 1, 3, 4)
                .reshape(128, 4, SLAB, TP))
            wv = wl[:, :, :, :, l0:l0 + L_LOC].reshape(
                IN_C, 9, OUT_C, L_LOC).transpose(0, 1, 3, 2)  # ci,combo,lp,c
            wA = np.ascontiguousarray(
                np.broadcast_to(wv[None], (2, IN_C, 9, L_LOC, OUT_C))
                .reshape(128, 9, L_LOC, OUT_C))
            m.update(xa=xa, wA=wA)
        in_maps.append(m)
    return in_maps


def kernel(x, weight, bias, gamma, beta, alpha, trace=False, variant=None):
    variant = variant or VARIANT
    x = np.asarray(x, np.float32)
    weight = np.asarray(weight, np.float32)
    bias = np.asarray(bias, np.float32)
    gamma = np.asarray(gamma, np.float32)
    beta = np.asarray(beta, np.float32)
    alpha = np.asarray(alpha, np.float32)

    key = "nc_" + variant
    if key not in _cache:
        _cache[key] = _build(variant)
    nc = _cache[key]
    in_maps = _prep(x, weight, bias, gamma, beta, alpha, variant)
    res = run_bass_kernel_spmd(nc, in_maps, list(range(NCORES)), trace=trace)
    kernel._last = res

    out = np.empty((B, OUT_C, L, T), np.float32)
    for r in range(NCORES):
        yo = np.asarray(res.results[r]["yo"]).astype(np.float32)
        l0 = r * L_LOC
        a6 = yo.reshape(2, 64, L_LOC, 2, 2, 256)
        if variant == "k128":
            # partition (bh, c); free (nb, b4r, t); b = bh*4 + nb*2 + b4r
            blk = a6.transpose(0, 3, 4, 1, 2, 5)
        else:
            # partition (nt, c); free (bh, b4r, t); b = bh*4 + nt*2 + b4r
            blk = a6.transpose(3, 0, 4, 1, 2, 5)
        out[:, :, l0:l0 + L_LOC, :] = blk.reshape(B, OUT_C, L_LOC, T)
    return out


# revision 4
# speedup vs baseline: 1.3864x; 1.3864x over previous
"""Trainium2 Bass kernel for LocalDenseConv2D + BatchNorm + PReLU.

Problem (hardcoded shapes):
  x:      (8, 64, 64, 256)  f32   (B, IN_C, L, T)
  weight: (576, 64, 64)     f32   (K = IN_C*9, OUT_C, OUT_L)  k = ci*9 + di*3 + dj
  bias:   (64, 64)          f32   (OUT_C, OUT_L)
  gamma, beta: (64,)        f32
  alpha:  (1,)              f32
  out:    (8, 64, 64, 256)  f32

Sharding: out_l across 8 cores (8 rows each), all batches per core.
Inputs are converted to bf16 on the host (matmul runs 1 cycle/row vs 2 for
f32r-HIGH, and input DMA halves).  Two conv structures:

  "k128": contraction packed to K=128 by stacking taps (di=0,di=1) on the
    two partition halves of a row-shifted x copy; taps di=2 run as K=64
    matmuls on partitions 0-63.  6 matmuls per (lp, bh, nb) region;
    98304 psum rows per core.  PSUM holds (bh*64+c) x (nb, b4, t).

  "quad4": baseline tap structure (K=64 over ci) but all four 64x64 PE
    quadrants active: row groups = batch half (bh), col groups = b4 pair
    (nt).  147456 rows but up to 4-way quadrant concurrency.

BatchNorm: per-partition bn_stats/bn_aggr -> (sum, sumsq) -> AllGather
[128,2] -> gather both partition halves per channel -> reduce -> exact
global mean/var.  BN-apply + PReLU is one scalar-engine Prelu with
per-partition scale/bias; the bias pass also uses Prelu (alpha=1) so only
one activation table set is ever loaded.  Output stored bf16, upcast on
host.
"""
import os
import sys
import numpy as np

if '/opt/trn_rl_repo' not in sys.path:
    sys.path.insert(0, '/opt/trn_rl_repo')

import concourse.bass as bass
import concourse.bacc as bacc
import concourse.mybir as mybir
import concourse.tile as tile
from concourse.bass_utils import run_bass_kernel_spmd

import ml_dtypes

BF16_NP = ml_dtypes.bfloat16
F32 = mybir.dt.float32
BF16 = mybir.dt.bfloat16
AF = mybir.ActivationFunctionType
ALU = mybir.AluOpType

B, IN_C, L, T = 8, 64, 64, 256
OUT_C, OUT_L = 64, 64
NCORES = 8
L_LOC = L // NCORES          # 8 out_l rows per core
SLAB = L_LOC + 2             # 10 x-rows incl. halo
TP = T + 2                   # padded t
EPS = 1e-5
N_PART = L_LOC * 2 * T * 2   # elems per partition per core = 8192
N_GLOBAL = B * L * T         # 131072

VARIANT = os.environ.get("KVARIANT", "k128")   # "k128" or "quad4"

_cache = {}


def _build_common_tail(nc, tc, cpool, ott, stats, gt, et, att, epst, yo):
    """stats merge -> collective -> global scale/shift -> final pass."""
    cc_in = nc.dram_tensor("cc_in", [128, 2], F32)
    cc_out = nc.dram_tensor("cc_out", [NCORES * 128, 2], F32,
                            addr_space="Shared")

    # local (mean, var) -> (sum, sumsq); off the critical path
    loc = cpool.tile([128, 2], F32)
    nc.vector.bn_aggr(loc[:], stats[:].rearrange("p a b c -> p (a b c)"))
    msq = cpool.tile([128, 1], F32)
    t2 = cpool.tile([128, 1], F32)
    ccs = cpool.tile([128, 2], F32)
    nc.vector.tensor_tensor(msq[:], loc[:, 0:1], loc[:, 0:1], ALU.mult)
    nc.vector.tensor_tensor(t2[:], loc[:, 1:2], msq[:], ALU.add)
    nc.vector.tensor_scalar_mul(ccs[:, 0:1], loc[:, 0:1], float(N_PART))
    nc.vector.tensor_scalar_mul(ccs[:, 1:2], t2[:], float(N_PART))
    nc.sync.dma_start(cc_in.ap(), ccs[:])
    nc.gpsimd.collective_compute(
        "AllGather", ALU.bypass,
        replica_groups=[list(range(NCORES))],
        ins=[cc_in[:]], outs=[cc_out[:]])

    # gather: partition p gets all 16 (rank, half) partials of its channel
    gm = cpool.tile([128, 2, 16], F32)
    src = cc_out.ap().rearrange("(r q c) s -> c s (r q)", r=NCORES, q=2, c=64)
    nc.sync.dma_start(gm[0:64, :, :], src)
    nc.scalar.dma_start(gm[64:128, :, :], src)

    tot = cpool.tile([128, 2], F32)
    nc.vector.tensor_reduce(tot[:], gm[:, :, :], axis=mybir.AxisListType.X,
                            op=ALU.add)
    mv = cpool.tile([128, 2], F32)       # (mean, E[x^2])
    nc.vector.tensor_scalar_mul(mv[:], tot[:], 1.0 / N_GLOBAL)
    msq2 = cpool.tile([128, 1], F32)
    var = cpool.tile([128, 1], F32)
    nc.vector.tensor_tensor(msq2[:], mv[:, 0:1], mv[:, 0:1], ALU.mult)
    nc.vector.tensor_tensor(var[:], mv[:, 1:2], msq2[:], ALU.subtract)
    std = cpool.tile([128, 1], F32)
    rstd = cpool.tile([128, 1], F32)
    sca = cpool.tile([128, 1], F32)
    shi = cpool.tile([128, 1], F32)
    nc.scalar.activation(std[:], var[:], AF.Sqrt, bias=epst[:])
    nc.vector.reciprocal(rstd[:], std[:])
    nc.vector.tensor_tensor(sca[:], gt[:], rstd[:], ALU.mult)
    nc.vector.tensor_tensor(shi[:], mv[:, 0:1], sca[:], ALU.mult)
    nc.vector.tensor_tensor(shi[:], et[:], shi[:], ALU.subtract)

    # fused BN-apply + PReLU + store (bf16)
    with tc.tile_pool(name="fp", bufs=3) as fpool:
        out_q = [nc.sync, nc.gpsimd]
        for lp in range(L_LOC):
            fo = fpool.tile([128, 1024], BF16, tag="fo")
            nc.scalar.activation(fo[:], ott[:, lp, :], AF.Prelu,
                                 bias=shi[:], scale=sca[:], alpha=att[:])
            out_q[lp % 2].dma_start(yo.ap()[:, lp, :], fo[:])


def _build(variant):
    nc = bacc.Bacc("TRN2", target_bir_lowering=False, debug=False,
                   num_devices=NCORES)
    bt_d = nc.dram_tensor("bt", [128, L_LOC], F32, kind="ExternalInput")
    g_d = nc.dram_tensor("g", [128, 1], F32, kind="ExternalInput")
    e_d = nc.dram_tensor("e", [128, 1], F32, kind="ExternalInput")
    a_d = nc.dram_tensor("a", [128, 1], F32, kind="ExternalInput")
    yo = nc.dram_tensor("yo", [128, L_LOC, 1024], BF16, kind="ExternalOutput")

    if variant == "k128":
        xa_d = nc.dram_tensor("xa", [128, B, SLAB, TP], BF16,
                              kind="ExternalInput")
        wA_d = nc.dram_tensor("wA", [128, 3, L_LOC, OUT_C], BF16,
                              kind="ExternalInput")
        wC_d = nc.dram_tensor("wC", [64, 3, L_LOC, OUT_C], BF16,
                              kind="ExternalInput")
    else:
        xa_d = nc.dram_tensor("xa", [128, 4, SLAB, TP], BF16,
                              kind="ExternalInput")
        wA_d = nc.dram_tensor("wA", [128, 9, L_LOC, OUT_C], BF16,
                              kind="ExternalInput")

    with tile.TileContext(nc) as tc:
        with (
            tc.tile_pool(name="const", bufs=1) as cpool,
            tc.tile_pool(name="xp", bufs=1) as xpool,
            tc.tile_pool(name="op", bufs=1) as opool,
            tc.tile_pool(name="ps", bufs=2, space="PSUM") as ppool,
        ):
            bt = cpool.tile([128, L_LOC], F32)
            gt = cpool.tile([128, 1], F32)
            et = cpool.tile([128, 1], F32)
            att = cpool.tile([128, 1], F32)
            onet = cpool.tile([128, 1], F32)
            epst = cpool.tile([128, 1], F32)
            nc.vector.memset(onet[:], 1.0)
            nc.vector.memset(epst[:], EPS)

            ott = opool.tile([128, L_LOC, 1024], F32)
            stats = cpool.tile([128, L_LOC, 2, 6], F32)

            if variant == "k128":
                wA = cpool.tile([128, 3, L_LOC, OUT_C], BF16)
                wC = cpool.tile([64, 3, L_LOC, OUT_C], BF16)
                xa = xpool.tile([128, B, SLAB, TP], BF16)
                nc.sync.dma_start(wA[:], wA_d.ap())
                nc.sync.dma_start(wC[:], wC_d.ap())
            else:
                wA = cpool.tile([128, 9, L_LOC, OUT_C], BF16)
                xa = xpool.tile([128, 4, SLAB, TP], BF16)
                nc.sync.dma_start(wA[:], wA_d.ap())
            nc.sync.dma_start(bt[:], bt_d.ap())
            nc.sync.dma_start(gt[:], g_d.ap())
            nc.sync.dma_start(et[:], e_d.ap())
            nc.sync.dma_start(att[:], a_d.ap())
            # x rows on two queues, in row order so early lps unblock first
            for s in range(SLAB):
                q = nc.sync if s % 2 == 0 else nc.scalar
                q.dma_start(xa[:, :, s, :], xa_d.ap()[:, :, s, :])

            # ---- conv ----
            for lp in range(L_LOC):
                pt = ppool.tile([128, 1024], F32, tag="pt")
                if variant == "k128":
                    # region (bh, nb): batches bh*4+2nb+{0,1}
                    for nb in range(2):
                        for mi in range(6):
                            first = mi == 0
                            last = mi == 5
                            for bh in range(2):
                                bsel = bh * 4 + 2 * nb
                                if mi < 3:   # K=128 pair (di=0,1), dj=mi
                                    j = mi
                                    lhsT = wA[:, j, lp, :]
                                    rhs = xa[:, bsel:bsel + 2, lp, j:j + T]
                                else:        # K=64 single di=2, dj=mi-3
                                    j = mi - 3
                                    lhsT = wC[:, j, lp, :]
                                    rhs = xa[0:64, bsel:bsel + 2, lp + 2,
                                             j:j + T]
                                nc.tensor.matmul(
                                    pt[bh * 64:(bh + 1) * 64,
                                       nb * 512:(nb + 1) * 512],
                                    lhsT, rhs, start=first, stop=last)
                else:
                    # quad4: psum partition = nt*64+c, free = bh*512+...
                    for combo in range(9):
                        di, dj = combo // 3, combo % 3
                        first = combo == 0
                        last = combo == 8
                        for bh in range(2):
                            for nt in range(2):
                                lhsT = wA[bh * 64:(bh + 1) * 64, combo, lp, :]
                                rhs = xa[bh * 64:(bh + 1) * 64,
                                         2 * nt:2 * nt + 2, lp + di,
                                         dj:dj + T]
                                nc.tensor.matmul(
                                    pt[nt * 64:(nt + 1) * 64,
                                       bh * 512:(bh + 1) * 512],
                                    lhsT, rhs, start=first, stop=last)

                # bias + copy to SBUF (Prelu with alpha=1 == identity)
                nc.scalar.activation(ott[:, lp, :], pt[:, :], AF.Prelu,
                                     bias=bt[:, lp:lp + 1], alpha=onet[:])
                for h in range(2):
                    nc.vector.bn_stats(stats[:, lp, h, :],
                                       pt[:, h * 512:(h + 1) * 512])

            _build_common_tail(nc, tc, cpool, ott, stats, gt, et, att,
                               epst, yo)
    nc.compile()
    return nc


def _prep(x, weight, bias, gamma, beta, alpha, variant):
    """Build per-core input maps (host-side shard + relayout, bf16)."""
    xpad = np.zeros((B, IN_C, L + 3, TP + 1), np.float32)
    xpad[:, :, 1:L + 1, 1:T + 1] = x
    xpad = xpad.astype(BF16_NP)
    wl = weight.reshape(IN_C, 3, 3, OUT_C, OUT_L).astype(BF16_NP)

    in_maps = []
    for r in range(NCORES):
        l0 = r * L_LOC
        m = {
            "bt": np.ascontiguousarray(
                np.tile(bias[:, l0:l0 + L_LOC], (2, 1))).astype(np.float32),
            "g": np.tile(gamma.reshape(-1, 1), (2, 1)).astype(np.float32),
            "e": np.tile(beta.reshape(-1, 1), (2, 1)).astype(np.float32),
            "a": np.full((128, 1), float(alpha[0]), np.float32),
        }
        if variant == "k128":
            slab = xpad[:, :, l0:l0 + SLAB + 1, :TP]  # (B, C, 11, 258)
            xa = np.empty((128, B, SLAB, TP), BF16_NP)
            xa[0:64] = slab[:, :, 0:SLAB].transpose(1, 0, 2, 3)
            xa[64:128] = slab[:, :, 1:SLAB + 1].transpose(1, 0, 2, 3)
            wv = wl[:, :, :, :, l0:l0 + L_LOC]  # (ci, di, dj, c, lp)
            wA = np.empty((128, 3, L_LOC, OUT_C), BF16_NP)
            wA[0:64] = wv[:, 0].transpose(0, 1, 3, 2)   # (ci, dj, lp, c)
            wA[64:128] = wv[:, 1].transpose(0, 1, 3, 2)
            wC = np.ascontiguousarray(wv[:, 2].transpose(0, 1, 3, 2))
            m.update(xa=xa, wA=wA, wC=np.ascontiguousarray(wC))
        else:
            slab = xpad[:, :, l0:l0 + SLAB, :TP]       # (B, C, 10, 258)
            xa = np.ascontiguousarray(
                slab.reshape(2, 4, IN_C, SLAB, TP).transpose(0, 2, 1, 3, 4)
                .reshape(128, 4, SLAB, TP))
            wv = wl[:, :, :, :, l0:l0 + L_LOC].reshape(
                IN_C, 9, OUT_C, L_LOC).transpose(0, 1, 3, 2)  # ci,combo,lp,c
            wA = np.ascontiguousarray(
                np.broadcast_to(wv[None], (2, IN_C, 9, L_LOC, OUT_C))
                .reshape(128, 9, L_LOC, OUT_C))
            m.update(xa=xa, wA=wA)
        in_maps.append(m)
    return in_maps


def kernel(x, weight, bias, gamma, beta, alpha, trace=False, variant=None):
    variant = variant or VARIANT
    x = np.asarray(x, np.float32)
    weight = np.asarray(weight, np.float32)
    bias = np.asarray(bias, np.float32)
    gamma = np.asarray(gamma, np.float32)
    beta = np.asarray(beta, np.float32)
    alpha = np.asarray(alpha, np.float32)

    key = "nc_" + variant
    if key not in _cache:
        _cache[key] = _build(variant)
    nc = _cache[key]
    in_maps = _prep(x, weight, bias, gamma, beta, alpha, variant)
    res = run_bass_kernel_spmd(nc, in_maps, list(range(NCORES)), trace=trace)
    kernel._last = res

    out = np.empty((B, OUT_C, L, T), np.float32)
    for r in range(NCORES):
        yo = np.asarray(res.results[r]["yo"]).astype(np.float32)
        l0 = r * L_LOC
        a6 = yo.reshape(2, 64, L_LOC, 2, 2, 256)
        if variant == "k128":
            # partition (bh, c); free (nb, b4r, t); b = bh*4 + nb*2 + b4r
            blk = a6.transpose(0, 3, 4, 1, 2, 5)
        else:
            # partition (nt, c); free (bh, b4r, t); b = bh*4 + nt*2 + b4r
            blk = a6.transpose(3, 0, 4, 1, 2, 5)
        out[:, :, l0:l0 + L_LOC, :] = blk.reshape(B, OUT_C, L_LOC, T)
    return out
